# revision 8
# baseline (speedup 1.0000x reference)
"""Trainium2 Bass kernel for nn_Attention_40475771798025.

Full attention layer: QKV projection + RoPE + GQA causal attention + output
projection. B=2, S=2048, D=4096, H=32 q-heads, KV=8 kv-heads, HD=128.

Sharding: head-parallel tensor parallelism across 8 cores. Core g owns kv-head
g (its 4 q-heads, 1 k-head, 1 v-head) for both batches. Weights are
pre-transposed on the host. All matmul operands are bf16 (numerics sim:
4.5e-3 rel err vs 2e-2 tolerance): bf16 streams as fast as f32r on the PE
but its Fast-Weight-Load halves LDWEIGHTS cost so weight-swapping matmul
streams run at ~stream-bound rate, and DMA/SBUF bytes halve.

Device kernel per core, per batch:
  A: qkv^T accumulated in PSUM over PAIRS of D-chunk passes (both passes'
     x/w staged, so each [128,1024] psum pair-tile takes all its matmuls
     before one f32 evacuation-add into the f32 acc; halves DVE/ACT
     evacuation work vs per-pass evacuation), evacuation alternates
     DVE/ACT. RoPE (rotation matmul on f32r acc + GPSIMD/DVE combine)
     writes bf16 into acc_bf; V transposed via PE into bf16 v_nat.
  B: per q-head, per 512-wide q-tile: scores^T k-pair tiles [128k, 2, 512q]
     (two matmuls into a 2-bank psum tile), causal mask bias on straddle
     tiles (DVE), ONE 1024-wide exp per pair (ACT, scale=1/sqrt(128)) to
     bf16 e-pairs, E@V and ones-denominator matmuls (bf16) trail by PIPE
     pairs, normalize with DVE reciprocal+mul writing bf16 att into acc_bf.
  C: partial out = attT^T @ woT (bf16), [128,1024] psum tiles evacuated
     bf16, written as bf16 partials summed on the host in float64.
"""
import sys
sys.path.insert(0, "/opt/trn_rl_repo")
import numpy as np
import ml_dtypes

BF16NP = ml_dtypes.bfloat16

B, S, D = 2, 2048, 4096
H, KV, HD = 32, 8, 128
REP = H // KV            # 4 q-heads per core
T = B * S                # 4096 flattened tokens
NCORES = 8
P = 128
QTW, KTW = 512, 128      # q-tile width (psum free dim), k-tile width
MQKV = REP + 2           # 6 m-tiles of 128: q0..q3, k, v
KSLOT = REP              # acc_bf slot for k
SCALE = 1.0 / float(np.sqrt(HD))

_nc = None


def _build_nc(reps=1):
    import concourse.bacc as bacc
    import concourse.mybir as mybir
    import concourse.tile as tile
    from contextlib import ExitStack

    F32 = mybir.dt.float32
    F32R = mybir.dt.float32r
    BF16 = mybir.dt.bfloat16

    nc = bacc.Bacc("TRN2")
    xT_d = nc.dram_tensor("xT", (D, T), BF16, kind="ExternalInput")
    wqkvT_d = nc.dram_tensor("wqkvT", (D, MQKV * P), BF16,
                             kind="ExternalInput")
    woT_d = nc.dram_tensor("woT", (REP * P, D), BF16, kind="ExternalInput")
    cdup_d = nc.dram_tensor("cdup", (P, T), F32, kind="ExternalInput")
    sdup_d = nc.dram_tensor("sdup", (P, T), F32, kind="ExternalInput")
    pt_d = nc.dram_tensor("pt", (P, P), F32, kind="ExternalInput")
    ones_d = nc.dram_tensor("ones", (P, P), BF16, kind="ExternalInput")
    ident_d = nc.dram_tensor("ident", (P, P), F32, kind="ExternalInput")
    maskd_d = nc.dram_tensor("maskd", (P, P), BF16,
                             kind="ExternalInput")
    out_d = nc.dram_tensor("out", (T, D), BF16, kind="ExternalOutput")

    NT = S // QTW            # 4 q-tiles per batch
    NKT = S // KTW           # 16 k-tiles per batch
    # D-contraction passes, processed in PAIRS sharing one psum accumulation
    CPAIRS = [(1, 2), (3, 3), (3, 3), (3, 3), (3, 3), (2, 3)]
    assert sum(a + b for a, b in CPAIRS) == D // P
    NWQ = 4                  # wo slices
    NQ = D // NWQ            # 1024 output cols per wo slice
    M_ORDER = [KSLOT, 0, KSLOT + 1, 1, 2, 3]   # k, q0, v, q1-3
    PIPE = 2                 # phase-B EV pipeline depth, in kt-pairs

    with tile.TileContext(nc) as tc, ExitStack() as top:
        persist = top.enter_context(tc.tile_pool(name="persist", bufs=1))
        accp = top.enter_context(tc.tile_pool(name="acc", bufs=1))
        abfp = top.enter_context(tc.tile_pool(name="abf", bufs=1))
        vnp = top.enter_context(tc.tile_pool(name="vnat", bufs=1))
        csp = top.enter_context(tc.tile_pool(name="cs", bufs=1))
        tmpp = top.enter_context(tc.tile_pool(name="tmp", bufs=2))
        xqp = top.enter_context(tc.tile_pool(name="xq", bufs=2))
        wqp = top.enter_context(tc.tile_pool(name="wql", bufs=2))
        psRot = top.enter_context(
            tc.tile_pool(name="psRot", bufs=1, space="PSUM"))

        pt_s = persist.tile([P, P], F32R)
        ones_s = persist.tile([P, P], BF16)
        ident_s = persist.tile([P, P], F32R)
        maskd_s = persist.tile([P, P], BF16)
        nc.scalar.dma_start(pt_s[:], pt_d[:].bitcast(F32R))
        nc.scalar.dma_start(ones_s[:], ones_d[:])
        nc.scalar.dma_start(ident_s[:], ident_d[:].bitcast(F32R))
        nc.scalar.dma_start(maskd_s[:], maskd_d[:])

        # acc: f32r psum-evacuation accumulator [128, m, S].
        # acc_bf: bf16 matmul operands for phases B/C: slots 0-3 = q heads
        # (roped, then overwritten by normalized attention), slot 4 = k.
        acc = accp.tile([P, MQKV, S], F32R)
        acc_bf = abfp.tile([P, REP + 1, S], BF16)
        v_nat = vnp.tile([P, NKT, HD], BF16)

        for _rep in range(reps):
          for b in range(B):
            bsl = slice(b * S, (b + 1) * S)

            cdup_b = csp.tile([P, NT, QTW], F32, tag="c")
            sdup_b = csp.tile([P, NT, QTW], F32, tag="s")
            nc.scalar.dma_start(
                cdup_b[:], cdup_d[:, bsl].rearrange("p (n q) -> p n q", q=QTW))
            nc.scalar.dma_start(
                sdup_b[:], sdup_d[:, bsl].rearrange("p (n q) -> p n q", q=QTW))

            def rope_m(m):
                """RoPE on acc slot m (f32r), writing bf16 into acc_bf."""
                for tt in range(NT):
                    tsl = slice(tt * QTW, (tt + 1) * QTW)
                    accsl = acc[:, m, tsl]
                    rps = psRot.tile([P, QTW], F32, tag="rot")
                    nc.tensor.matmul(rps[:], lhsT=pt_s[:], rhs=accsl,
                                     start=True, stop=True)
                    t1 = tmpp.tile([P, QTW], F32, tag="t1")
                    t2 = tmpp.tile([P, QTW], F32, tag="t2")
                    nc.gpsimd.tensor_mul(t1[:], accsl.bitcast(F32),
                                         cdup_b[:, tt, :])
                    nc.vector.tensor_mul(t2[:], rps[:], sdup_b[:, tt, :])
                    nc.vector.tensor_add(acc_bf[:, m, tsl], t1[:], t2[:])

            # ---- phase A: projections + rope(k, q0) + v transpose ----
            with ExitStack() as actx:
                psA = actx.enter_context(
                    tc.tile_pool(name="psA", bufs=2, space="PSUM"))
                psVT = actx.enter_context(
                    tc.tile_pool(name="psVT", bufs=2, space="PSUM"))

                c_off = 0
                for dq, (cn0, cn1) in enumerate(CPAIRS):
                    c_n = cn0 + cn1
                    dsl = slice(c_off * P, (c_off + c_n) * P)
                    c_off += c_n
                    xq = xqp.tile([P, c_n, S], BF16, tag="xq",
                                  padded_shape=[P, 6, S])
                    xsrc = (xT_d[dsl, bsl]
                            .rearrange("(c p) t -> p c t", p=P))
                    wql = wqp.tile([P, c_n, MQKV * P], BF16, tag="wql",
                                   padded_shape=[P, 6, MQKV * P])
                    wsrc = (wqkvT_d[dsl, :]
                            .rearrange("(c p) m -> p c m", p=P))
                    if dq == 0:
                        # k-head weights (cols 512:640) land first and loads
                        # are chunked so the first matmuls start early
                        nc.sync.dma_start(wql[:, :, 3 * P:], wsrc[:, :, 3 * P:])
                        nc.sync.dma_start(wql[:, :, :3 * P], wsrc[:, :, :3 * P])
                        for cc in range(c_n):
                            nc.sync.dma_start(xq[:, cc], xsrc[:, cc])
                    else:
                        nc.sync.dma_start(wql[:], wsrc[:])
                        nc.sync.dma_start(xq[:, :cn0], xsrc[:, :cn0])
                        nc.sync.dma_start(xq[:, cn0:], xsrc[:, cn0:])
                    for m in M_ORDER:
                        for tp in range(NT // 2):   # tt pairs share a psum
                            ps = psA.tile([P, 2 * QTW], F32, tag="pa")
                            for half in range(2):
                                tt = tp * 2 + half
                                for c in range(c_n):
                                    nc.tensor.matmul(
                                        ps[:, half * QTW:(half + 1) * QTW],
                                        lhsT=wql[:, c, m * P:(m + 1) * P],
                                        rhs=xq[:, c,
                                               tt * QTW:(tt + 1) * QTW],
                                        start=(c == 0), stop=(c == c_n - 1))
                            accsl = acc[:, m,
                                        tp * 2 * QTW:(tp + 1) * 2 * QTW]
                            if dq == 0:
                                nc.scalar.copy(accsl, ps[:])
                            else:
                                nc.vector.tensor_add(
                                    accsl, accsl.bitcast(F32), ps[:])
                        if dq == len(CPAIRS) - 1:
                            if m in (KSLOT, 0):
                                rope_m(m)   # k and q0; q1-3 roped in phase B
                            elif m == KSLOT + 1:
                                for c in range(NKT):  # v -> natural layout
                                    tps = psVT.tile([P, P], F32R, tag="vt")
                                    nc.tensor.transpose(
                                        tps[:], acc[:, m, c * P:(c + 1) * P],
                                        ident_s[:])
                                    if c % 2 == 0:
                                        nc.scalar.copy(v_nat[:, c, :],
                                                       tps[:].bitcast(F32))
                                    else:
                                        nc.vector.tensor_copy(
                                            v_nat[:, c, :],
                                            tps[:].bitcast(F32))

            # ---- phase B: attention (+ trailing rope), then phase C ----
            with ExitStack() as bat:
                ep = bat.enter_context(tc.tile_pool(name="e", bufs=PIPE + 4))
                dap = bat.enter_context(tc.tile_pool(name="dac", bufs=2))
                rp = bat.enter_context(tc.tile_pool(name="rec", bufs=2))
                psS = bat.enter_context(
                    tc.tile_pool(name="psS", bufs=2, space="PSUM"))
                psO = bat.enter_context(
                    tc.tile_pool(name="psO", bufs=2, space="PSUM"))
                psD = bat.enter_context(
                    tc.tile_pool(name="psD", bufs=1, space="PSUM"))
                wop = bat.enter_context(tc.tile_pool(name="wo", bufs=2))
                obp = bat.enter_context(tc.tile_pool(name="ob", bufs=3))
                for h in range(REP):
                    if h + 1 < REP:
                        rope_m(h + 1)   # rope next head under this block
                    for qt in range(NT):
                        qsl = slice(qt * QTW, (qt + 1) * QTW)
                        nkt = (qt + 1) * (QTW // KTW)
                        nkp = nkt // 2  # kt-pairs

                        def w0_of(kt, qt=qt):
                            # straddle tiles: columns q < w0 have k > q
                            return max(0, (kt - qt * (QTW // KTW))) * KTW

                        ps_o = psO.tile([P, QTW], F32, tag="o")
                        d_acc = dap.tile([P, QTW], BF16, tag="d")
                        pend = []  # [(e_pair, kp), ...]

                        def flush(upto, ps_o=ps_o, d_acc=d_acc, nkt=nkt,
                                  pend=pend):
                            while len(pend) > upto:
                                pe, pkp = pend.pop(0)
                                for j in range(2):
                                    pkt = pkp * 2 + j
                                    w0 = w0_of(pkt)
                                    nc.tensor.matmul(
                                        ps_o[:, w0:],
                                        lhsT=v_nat[:, pkt, :],
                                        rhs=pe[:, j, w0:],
                                        start=(pkt == 0),
                                        stop=(pkt == nkt - 1),
                                        skip_group_check=True)
                                    if pkt == 0:
                                        nc.vector.tensor_copy(
                                            d_acc[:], pe[:, 0, :])
                                    else:
                                        nc.vector.tensor_add(
                                            d_acc[:, w0:], d_acc[:, w0:],
                                            pe[:, j, w0:])

                        for kp in range(nkp):
                            ps_s = psS.tile([P, 2, QTW], F32, tag="s")
                            straddle = kp >= 2 * qt
                            for j in range(2):
                                kt = kp * 2 + j
                                w0 = w0_of(kt)
                                nc.tensor.matmul(
                                    ps_s[:, j, w0:],
                                    lhsT=acc_bf[:, KSLOT,
                                                kt * KTW:(kt + 1) * KTW],
                                    rhs=acc_bf[:, h,
                                               qt * QTW + w0:(qt + 1) * QTW],
                                    start=True, stop=True)
                                if straddle:
                                    # triangular bias on the diagonal block
                                    nc.vector.tensor_add(
                                        ps_s[:, j, w0:w0 + KTW],
                                        ps_s[:, j, w0:w0 + KTW],
                                        maskd_s[:])
                            e = ep.tile([P, 2, QTW], BF16, tag="e")
                            if straddle:
                                for j in range(2):
                                    w0 = w0_of(kp * 2 + j)
                                    nc.scalar.activation(
                                        e[:, j, w0:], ps_s[:, j, w0:],
                                        mybir.ActivationFunctionType.Exp,
                                        scale=SCALE)
                            else:
                                nc.scalar.activation(
                                    e[:].rearrange("p a b -> p (a b)"),
                                    ps_s[:].rearrange("p a b -> p (a b)"),
                                    mybir.ActivationFunctionType.Exp,
                                    scale=SCALE)
                            pend.append((e, kp))
                            flush(PIPE)
                        flush(0)
                        ps_d = psD.tile([P, QTW], F32, tag="d")
                        nc.tensor.matmul(ps_d[:], lhsT=ones_s[:],
                                         rhs=d_acc[:], start=True, stop=True)
                        rec = rp.tile([P, QTW], F32, tag="rec")
                        nc.vector.reciprocal(rec[:], ps_d[:])
                        # overwrite q slot h with normalized attention out
                        nc.vector.tensor_mul(acc_bf[:, h, qsl], ps_o[:],
                                             rec[:])

                # ---- phase C: output projection (partial) ----
                for nq in range(NWQ):
                    nsl = slice(nq * NQ, (nq + 1) * NQ)
                    woh = wop.tile([P, REP, NQ], BF16, tag="wo")
                    nc.scalar.dma_start(
                        woh[:],
                        woT_d[:, nsl].rearrange("(h p) n -> p h n", p=P))
                    for tt in range(S // P):
                        ps = psS.tile([P, 2, QTW], F32, tag="s")
                        for half in range(NQ // QTW):
                            for h in range(REP):
                                nc.tensor.matmul(
                                    ps[:, half, :],
                                    lhsT=acc_bf[:, h, tt * P:(tt + 1) * P],
                                    rhs=woh[:, h,
                                            half * QTW:(half + 1) * QTW],
                                    start=(h == 0), stop=(h == REP - 1))
                        ob = obp.tile([P, NQ], BF16, tag="ob")
                        if tt % 2 == 0:
                            nc.scalar.copy(
                                ob[:], ps[:].rearrange("p a b -> p (a b)"))
                        else:
                            nc.vector.tensor_copy(
                                ob[:], ps[:].rearrange("p a b -> p (a b)"))
                        nc.sync.dma_start(
                            out_d[b * S + tt * P:b * S + (tt + 1) * P, nsl],
                            ob[:])
    nc.compile()
    return nc


def get_nc():
    global _nc
    if _nc is None:
        _nc = _build_nc()
    return _nc


def make_in_maps(x, freqs_cos, freqs_sin, wq, wk, wv, wo):
    """Host-side prep: transposes, rope tables, masks, per-core weight shards."""
    x = np.ascontiguousarray(x, np.float32)
    fc = np.asarray(freqs_cos, np.float32)
    fs = np.asarray(freqs_sin, np.float32)
    wq = np.asarray(wq, np.float32)
    wk = np.asarray(wk, np.float32)
    wv = np.asarray(wv, np.float32)
    wo = np.asarray(wo, np.float32)

    xT = np.ascontiguousarray(x.reshape(T, D).T.astype(BF16NP))
    cdup = np.ascontiguousarray(np.tile(np.repeat(fc.T, 2, axis=0), (1, B)))
    sdup = np.ascontiguousarray(np.tile(np.repeat(fs.T, 2, axis=0), (1, B)))
    prot = np.zeros((P, P), np.float32)
    for i in range(P // 2):
        prot[2 * i, 2 * i + 1] = -1.0
        prot[2 * i + 1, 2 * i] = 1.0
    pt = np.ascontiguousarray(prot.T)
    ones = np.ones((P, P), BF16NP)
    ident = np.eye(P, dtype=np.float32)
    ki = np.arange(P)[:, None]
    qi = np.arange(P)[None, :]
    maskd = np.ascontiguousarray(
        np.where(ki > qi, -1e9, 0.0).astype(BF16NP))  # [128, 128]

    in_maps = []
    for g in range(NCORES):
        wq_g = wq[g * REP * HD:(g + 1) * REP * HD]
        wk_g = wk[g * HD:(g + 1) * HD]
        wv_g = wv[g * HD:(g + 1) * HD]
        wqkvT = np.ascontiguousarray(
            np.concatenate([wq_g, wk_g, wv_g], 0).T.astype(BF16NP))
        woT = np.ascontiguousarray(
            wo[:, g * REP * HD:(g + 1) * REP * HD].T.astype(BF16NP))
        in_maps.append({
            "xT": xT, "wqkvT": wqkvT, "woT": woT,
            "cdup": cdup, "sdup": sdup, "pt": pt, "ones": ones,
            "ident": ident, "maskd": maskd,
        })
    return in_maps


def kernel(x, freqs_cos, freqs_sin, wq, wk, wv, wo):
    from concourse.bass_utils import run_bass_kernel_spmd
    nc = get_nc()
    in_maps = make_in_maps(x, freqs_cos, freqs_sin, wq, wk, wv, wo)
    res = run_bass_kernel_spmd(nc, in_maps, core_ids=list(range(NCORES)))
    out = np.zeros((T, D), np.float64)
    for r in res.results:
        out += r["out"].astype(np.float64)
    return out.astype(np.float32).reshape(B, S, D)


# revision 11
# speedup vs baseline: 1.0219x; 1.0219x over previous
"""Trainium2 Bass kernel for nn_Attention_40475771798025.

Full attention layer: QKV projection + RoPE + GQA causal attention + output
projection. B=2, S=2048, D=4096, H=32 q-heads, KV=8 kv-heads, HD=128.

Sharding: head-parallel tensor parallelism across 8 cores. Core g owns kv-head
g (its 4 q-heads, 1 k-head, 1 v-head) for both batches. Weights are
pre-transposed on the host. All matmul operands are bf16 (numerics sim:
4.5e-3 rel err vs 2e-2 tolerance): bf16 streams as fast as f32r on the PE
but its Fast-Weight-Load halves LDWEIGHTS cost so weight-swapping matmul
streams run at ~stream-bound rate, and DMA/SBUF bytes halve.

Device kernel per core, per batch:
  A: qkv^T accumulated in PSUM over PAIRS of D-chunk passes (both passes'
     x/w staged, so each [128,1024] psum pair-tile takes all its matmuls
     before one f32 evacuation-add into the f32 acc; halves DVE/ACT
     evacuation work vs per-pass evacuation), evacuation alternates
     DVE/ACT. RoPE (rotation matmul on f32r acc + GPSIMD/DVE combine)
     writes bf16 into acc_bf; V transposed via PE into bf16 v_nat.
  B: per q-head, per 512-wide q-tile: scores^T k-pair tiles [128k, 2, 512q]
     (two matmuls into a 2-bank psum tile), causal mask bias on straddle
     tiles (DVE), ONE 1024-wide exp per pair (ACT, scale=1/sqrt(128)) to
     bf16 e-pairs, E@V and ones-denominator matmuls (bf16) trail by PIPE
     pairs, normalize with DVE reciprocal+mul writing bf16 att into acc_bf.
  C: partial out = attT^T @ woT (bf16), [128,1024] psum tiles evacuated
     bf16, written as bf16 partials summed on the host in float64.
"""
import sys
sys.path.insert(0, "/opt/trn_rl_repo")
import numpy as np
import ml_dtypes

BF16NP = ml_dtypes.bfloat16

B, S, D = 2, 2048, 4096
H, KV, HD = 32, 8, 128
REP = H // KV            # 4 q-heads per core
T = B * S                # 4096 flattened tokens
NCORES = 8
P = 128
QTW, KTW = 512, 128      # q-tile width (psum free dim), k-tile width
MQKV = REP + 2           # 6 m-tiles of 128: q0..q3, k, v
KSLOT = REP              # acc_bf slot for k
SCALE = 1.0 / float(np.sqrt(HD))

_nc = None


def _build_nc(reps=1):
    import concourse.bacc as bacc
    import concourse.mybir as mybir
    import concourse.tile as tile
    from contextlib import ExitStack

    F32 = mybir.dt.float32
    F32R = mybir.dt.float32r
    BF16 = mybir.dt.bfloat16

    nc = bacc.Bacc("TRN2")
    xT_d = nc.dram_tensor("xT", (D, T), BF16, kind="ExternalInput")
    wqkvT_d = nc.dram_tensor("wqkvT", (D, MQKV * P), BF16,
                             kind="ExternalInput")
    woT_d = nc.dram_tensor("woT", (REP * P, D), BF16, kind="ExternalInput")
    cdup_d = nc.dram_tensor("cdup", (P, T), F32, kind="ExternalInput")
    sdup_d = nc.dram_tensor("sdup", (P, T), F32, kind="ExternalInput")
    pt_d = nc.dram_tensor("pt", (P, P), F32, kind="ExternalInput")
    ones_d = nc.dram_tensor("ones", (P, P), BF16, kind="ExternalInput")
    ident_d = nc.dram_tensor("ident", (P, P), F32, kind="ExternalInput")
    maskd_d = nc.dram_tensor("maskd", (P, P), BF16,
                             kind="ExternalInput")
    out_d = nc.dram_tensor("out", (T, D), BF16, kind="ExternalOutput")

    NT = S // QTW            # 4 q-tiles per batch
    NKT = S // KTW           # 16 k-tiles per batch
    # D-contraction passes, processed in PAIRS sharing one psum accumulation
    CPAIRS = [(1, 2), (3, 3), (3, 3), (3, 3), (3, 3), (2, 3)]
    assert sum(a + b for a, b in CPAIRS) == D // P
    NWQ = 4                  # wo slices
    NQ = D // NWQ            # 1024 output cols per wo slice
    M_ORDER = [KSLOT, 0, KSLOT + 1, 1, 2, 3]   # k, q0, v, q1-3
    PIPE = 2                 # phase-B EV pipeline depth, in kt-pairs

    with tile.TileContext(nc) as tc, ExitStack() as top:
        persist = top.enter_context(tc.tile_pool(name="persist", bufs=1))
        accp = top.enter_context(tc.tile_pool(name="acc", bufs=1))
        abfp = top.enter_context(tc.tile_pool(name="abf", bufs=1))
        vnp = top.enter_context(tc.tile_pool(name="vnat", bufs=1))
        csp = top.enter_context(tc.tile_pool(name="cs", bufs=1))
        tmpp = top.enter_context(tc.tile_pool(name="tmp", bufs=2))
        xqp = top.enter_context(tc.tile_pool(name="xq", bufs=2))
        wqp = top.enter_context(tc.tile_pool(name="wql", bufs=2))
        psRot = top.enter_context(
            tc.tile_pool(name="psRot", bufs=1, space="PSUM"))

        pt_s = persist.tile([P, P], F32R)
        ones_s = persist.tile([P, P], BF16)
        ident_s = persist.tile([P, P], F32R)
        maskd_s = persist.tile([P, P], BF16)
        nc.scalar.dma_start(pt_s[:], pt_d[:].bitcast(F32R))
        nc.scalar.dma_start(ones_s[:], ones_d[:])
        nc.scalar.dma_start(ident_s[:], ident_d[:].bitcast(F32R))
        nc.scalar.dma_start(maskd_s[:], maskd_d[:])

        # acc: f32r psum-evacuation accumulator [128, m, S].
        # acc_bf: bf16 matmul operands for phases B/C: slots 0-3 = q heads
        # (roped, then overwritten by normalized attention), slot 4 = k.
        acc = accp.tile([P, MQKV, S], F32R)
        acc_bf = abfp.tile([P, REP + 1, S], BF16)
        v_nat = vnp.tile([P, NKT, HD], BF16)

        for _rep in range(reps):
          for b in range(B):
            bsl = slice(b * S, (b + 1) * S)

            cdup_b = csp.tile([P, NT, QTW], F32, tag="c")
            sdup_b = csp.tile([P, NT, QTW], F32, tag="s")
            nc.scalar.dma_start(
                cdup_b[:], cdup_d[:, bsl].rearrange("p (n q) -> p n q", q=QTW))
            nc.scalar.dma_start(
                sdup_b[:], sdup_d[:, bsl].rearrange("p (n q) -> p n q", q=QTW))

            def rope_m(m):
                """RoPE on acc slot m (f32r), writing bf16 into acc_bf."""
                for tt in range(NT):
                    tsl = slice(tt * QTW, (tt + 1) * QTW)
                    accsl = acc[:, m, tsl]
                    rps = psRot.tile([P, QTW], F32, tag="rot")
                    nc.tensor.matmul(rps[:], lhsT=pt_s[:], rhs=accsl,
                                     start=True, stop=True)
                    t1 = tmpp.tile([P, QTW], F32, tag="t1")
                    t2 = tmpp.tile([P, QTW], F32, tag="t2")
                    nc.gpsimd.tensor_mul(t1[:], accsl.bitcast(F32),
                                         cdup_b[:, tt, :])
                    nc.vector.tensor_mul(t2[:], rps[:], sdup_b[:, tt, :])
                    nc.vector.tensor_add(acc_bf[:, m, tsl], t1[:], t2[:])

            # ---- phase A: projections + rope(k, q0) + v transpose ----
            with ExitStack() as actx:
                psA = actx.enter_context(
                    tc.tile_pool(name="psA", bufs=2, space="PSUM"))
                psVT = actx.enter_context(
                    tc.tile_pool(name="psVT", bufs=2, space="PSUM"))

                c_off = 0
                for dq, (cn0, cn1) in enumerate(CPAIRS):
                    c_n = cn0 + cn1
                    dsl = slice(c_off * P, (c_off + c_n) * P)
                    c_off += c_n
                    xq = xqp.tile([P, c_n, S], BF16, tag="xq",
                                  padded_shape=[P, 6, S])
                    xsrc = (xT_d[dsl, bsl]
                            .rearrange("(c p) t -> p c t", p=P))
                    wql = wqp.tile([P, c_n, MQKV * P], BF16, tag="wql",
                                   padded_shape=[P, 6, MQKV * P])
                    wsrc = (wqkvT_d[dsl, :]
                            .rearrange("(c p) m -> p c m", p=P))
                    if dq == 0:
                        # k-head weights (cols 512:640) land first and loads
                        # are chunked so the first matmuls start early
                        nc.sync.dma_start(wql[:, :, 3 * P:], wsrc[:, :, 3 * P:])
                        nc.sync.dma_start(wql[:, :, :3 * P], wsrc[:, :, :3 * P])
                        for cc in range(c_n):
                            nc.sync.dma_start(xq[:, cc], xsrc[:, cc])
                    else:
                        nc.sync.dma_start(wql[:], wsrc[:])
                        nc.sync.dma_start(xq[:, :cn0], xsrc[:, :cn0])
                        nc.sync.dma_start(xq[:, cn0:], xsrc[:, cn0:])
                    for m in M_ORDER:
                        for tp in range(NT // 2):   # tt pairs share a psum
                            ps = psA.tile([P, 2 * QTW], F32, tag="pa")
                            for half in range(2):
                                tt = tp * 2 + half
                                for c in range(c_n):
                                    nc.tensor.matmul(
                                        ps[:, half * QTW:(half + 1) * QTW],
                                        lhsT=wql[:, c, m * P:(m + 1) * P],
                                        rhs=xq[:, c,
                                               tt * QTW:(tt + 1) * QTW],
                                        start=(c == 0), stop=(c == c_n - 1))
                            accsl = acc[:, m,
                                        tp * 2 * QTW:(tp + 1) * 2 * QTW]
                            if dq == 0:
                                nc.scalar.copy(accsl, ps[:])
                            else:
                                nc.vector.tensor_add(
                                    accsl, accsl.bitcast(F32), ps[:])
                        if dq == len(CPAIRS) - 1:
                            if m in (KSLOT, 0):
                                rope_m(m)   # k and q0; q1-3 roped in phase B
                            elif m == KSLOT + 1:
                                for c in range(NKT):  # v -> natural layout
                                    tps = psVT.tile([P, P], F32R, tag="vt")
                                    nc.tensor.transpose(
                                        tps[:], acc[:, m, c * P:(c + 1) * P],
                                        ident_s[:])
                                    if c % 2 == 0:
                                        nc.scalar.copy(v_nat[:, c, :],
                                                       tps[:].bitcast(F32))
                                    else:
                                        nc.vector.tensor_copy(
                                            v_nat[:, c, :],
                                            tps[:].bitcast(F32))

            # ---- phase B: attention (+ trailing rope), then phase C ----
            with ExitStack() as bat:
                ep = bat.enter_context(tc.tile_pool(name="e", bufs=PIPE + 4))
                dap = bat.enter_context(tc.tile_pool(name="dac", bufs=2))
                rp = bat.enter_context(tc.tile_pool(name="rec", bufs=2))
                psS = bat.enter_context(
                    tc.tile_pool(name="psS", bufs=2, space="PSUM"))
                psO = bat.enter_context(
                    tc.tile_pool(name="psO", bufs=2, space="PSUM"))
                psD = bat.enter_context(
                    tc.tile_pool(name="psD", bufs=1, space="PSUM"))
                wop = bat.enter_context(tc.tile_pool(name="wo", bufs=2))
                obp = bat.enter_context(tc.tile_pool(name="ob", bufs=3))
                fin = []   # deferred per-qt finalizers (run one qt later
                           # so the DVE d_acc chain never stalls the PE)

                def drain_fin(upto=0):
                    while len(fin) > upto:
                        fin.pop(0)()

                for h in range(REP):
                    if h + 1 < REP:
                        rope_m(h + 1)   # rope next head under this block
                    for qt in range(NT):
                        qsl = slice(qt * QTW, (qt + 1) * QTW)
                        nkt = (qt + 1) * (QTW // KTW)
                        nkp = nkt // 2  # kt-pairs

                        def w0_of(kt, qt=qt):
                            # straddle tiles: columns q < w0 have k > q
                            return max(0, (kt - qt * (QTW // KTW))) * KTW

                        ps_o = psO.tile([P, QTW], F32, tag="o")
                        d_acc = dap.tile([P, QTW], BF16, tag="d")
                        pend = []  # [(e_pair, kp), ...]

                        def flush(upto, ps_o=ps_o, d_acc=d_acc, nkt=nkt,
                                  pend=pend):
                            while len(pend) > upto:
                                pe, pkp = pend.pop(0)
                                for j in range(2):
                                    pkt = pkp * 2 + j
                                    w0 = w0_of(pkt)
                                    nc.tensor.matmul(
                                        ps_o[:, w0:],
                                        lhsT=v_nat[:, pkt, :],
                                        rhs=pe[:, j, w0:],
                                        start=(pkt == 0),
                                        stop=(pkt == nkt - 1),
                                        skip_group_check=True)
                                    if pkt == 0:
                                        nc.vector.tensor_copy(
                                            d_acc[:], pe[:, 0, :])
                                    else:
                                        nc.vector.tensor_add(
                                            d_acc[:, w0:], d_acc[:, w0:],
                                            pe[:, j, w0:])

                        for kp in range(nkp):
                            ps_s = psS.tile([P, 2, QTW], F32, tag="s")
                            straddle = kp >= 2 * qt
                            for j in range(2):
                                kt = kp * 2 + j
                                w0 = w0_of(kt)
                                nc.tensor.matmul(
                                    ps_s[:, j, w0:],
                                    lhsT=acc_bf[:, KSLOT,
                                                kt * KTW:(kt + 1) * KTW],
                                    rhs=acc_bf[:, h,
                                               qt * QTW + w0:(qt + 1) * QTW],
                                    start=True, stop=True)
                                if straddle:
                                    # triangular bias on the diagonal block
                                    nc.vector.tensor_add(
                                        ps_s[:, j, w0:w0 + KTW],
                                        ps_s[:, j, w0:w0 + KTW],
                                        maskd_s[:])
                            e = ep.tile([P, 2, QTW], BF16, tag="e")
                            if straddle:
                                for j in range(2):
                                    w0 = w0_of(kp * 2 + j)
                                    nc.scalar.activation(
                                        e[:, j, w0:], ps_s[:, j, w0:],
                                        mybir.ActivationFunctionType.Exp,
                                        scale=SCALE)
                            else:
                                nc.scalar.activation(
                                    e[:].rearrange("p a b -> p (a b)"),
                                    ps_s[:].rearrange("p a b -> p (a b)"),
                                    mybir.ActivationFunctionType.Exp,
                                    scale=SCALE)
                            pend.append((e, kp))
                            if kp == min(1, nkp - 1):
                                drain_fin()   # prev qt's finalize runs here,
                                              # after this qt's first scores
                            flush(PIPE)
                        flush(0)

                        def finalize(ps_o=ps_o, d_acc=d_acc, h=h, qsl=qsl):
                            ps_d = psD.tile([P, QTW], F32, tag="d")
                            nc.tensor.matmul(ps_d[:], lhsT=ones_s[:],
                                             rhs=d_acc[:],
                                             start=True, stop=True)
                            rec = rp.tile([P, QTW], F32, tag="rec")
                            nc.vector.reciprocal(rec[:], ps_d[:])
                            # overwrite q slot h with normalized attention
                            nc.vector.tensor_mul(acc_bf[:, h, qsl], ps_o[:],
                                                 rec[:])
                        fin.append(finalize)
                drain_fin()

                # ---- phase C: output projection (partial) ----
                for nq in range(NWQ):
                    nsl = slice(nq * NQ, (nq + 1) * NQ)
                    woh = wop.tile([P, REP, NQ], BF16, tag="wo")
                    nc.scalar.dma_start(
                        woh[:],
                        woT_d[:, nsl].rearrange("(h p) n -> p h n", p=P))
                    for tt in range(S // P):
                        ps = psS.tile([P, 2, QTW], F32, tag="s")
                        for half in range(NQ // QTW):
                            for h in range(REP):
                                nc.tensor.matmul(
                                    ps[:, half, :],
                                    lhsT=acc_bf[:, h, tt * P:(tt + 1) * P],
                                    rhs=woh[:, h,
                                            half * QTW:(half + 1) * QTW],
                                    start=(h == 0), stop=(h == REP - 1))
                        ob = obp.tile([P, NQ], BF16, tag="ob")
                        if tt % 2 == 0:
                            nc.scalar.copy(
                                ob[:], ps[:].rearrange("p a b -> p (a b)"))
                        else:
                            nc.vector.tensor_copy(
                                ob[:], ps[:].rearrange("p a b -> p (a b)"))
                        nc.sync.dma_start(
                            out_d[b * S + tt * P:b * S + (tt + 1) * P, nsl],
                            ob[:])
    nc.compile()
    return nc


def get_nc():
    global _nc
    if _nc is None:
        _nc = _build_nc()
    return _nc


def make_in_maps(x, freqs_cos, freqs_sin, wq, wk, wv, wo):
    """Host-side prep: transposes, rope tables, masks, per-core weight shards."""
    x = np.ascontiguousarray(x, np.float32)
    fc = np.asarray(freqs_cos, np.float32)
    fs = np.asarray(freqs_sin, np.float32)
    wq = np.asarray(wq, np.float32)
    wk = np.asarray(wk, np.float32)
    wv = np.asarray(wv, np.float32)
    wo = np.asarray(wo, np.float32)

    xT = np.ascontiguousarray(x.reshape(T, D).T.astype(BF16NP))
    cdup = np.ascontiguousarray(np.tile(np.repeat(fc.T, 2, axis=0), (1, B)))
    sdup = np.ascontiguousarray(np.tile(np.repeat(fs.T, 2, axis=0), (1, B)))
    prot = np.zeros((P, P), np.float32)
    for i in range(P // 2):
        prot[2 * i, 2 * i + 1] = -1.0
        prot[2 * i + 1, 2 * i] = 1.0
    pt = np.ascontiguousarray(prot.T)
    ones = np.ones((P, P), BF16NP)
    ident = np.eye(P, dtype=np.float32)
    ki = np.arange(P)[:, None]
    qi = np.arange(P)[None, :]
    maskd = np.ascontiguousarray(
        np.where(ki > qi, -1e9, 0.0).astype(BF16NP))  # [128, 128]

    in_maps = []
    for g in range(NCORES):
        wq_g = wq[g * REP * HD:(g + 1) * REP * HD]
        wk_g = wk[g * HD:(g + 1) * HD]
        wv_g = wv[g * HD:(g + 1) * HD]
        wqkvT = np.ascontiguousarray(
            np.concatenate([wq_g, wk_g, wv_g], 0).T.astype(BF16NP))
        woT = np.ascontiguousarray(
            wo[:, g * REP * HD:(g + 1) * REP * HD].T.astype(BF16NP))
        in_maps.append({
            "xT": xT, "wqkvT": wqkvT, "woT": woT,
            "cdup": cdup, "sdup": sdup, "pt": pt, "ones": ones,
            "ident": ident, "maskd": maskd,
        })
    return in_maps


def kernel(x, freqs_cos, freqs_sin, wq, wk, wv, wo):
    from concourse.bass_utils import run_bass_kernel_spmd
    nc = get_nc()
    in_maps = make_in_maps(x, freqs_cos, freqs_sin, wq, wk, wv, wo)
    res = run_bass_kernel_spmd(nc, in_maps, core_ids=list(range(NCORES)))
    out = np.zeros((T, D), np.float64)
    for r in res.results:
        out += r["out"].astype(np.float64)
    return out.astype(np.float32).reshape(B, S, D)
